# revision 16
# baseline (speedup 1.0000x reference)
"""GRU kernel for Trainium2 (8 NeuronCores, data-parallel over batch).

Problem: nn_GRU — X [256, 512, 128] f32, W_z/W_r/W_c [256, 384], b_* [256].
Output: h_history [512, 256, 256] f32.

Sharding: batch 256 -> 8 cores x 32. Each core runs an independent GRU
recurrence over its batch shard; weights replicated; no collectives.

Design (latency-oriented: the 512-step recurrence is serial):
  - bf16 matmul operands, fp32 PSUM accumulation.
  - h_t is carried as the pair (v_t, mu_t) with v = z*c, mu = (z-1)*h_prev,
    h = v - mu. The recurrence matmuls consume v and mu directly (mu through
    negated weight copies), so the h-combine leaves the critical path.
  - r-gate sigmoid is a single fused custom DVE op (deg-7 odd minimax of
    sigma-0.5; r preacts stay within its fit range), followed by one
    scalar_tensor_tensor for rh = (r'+0.5)*h. The Activation engine only
    handles the z-sigmoid and candidate tanh (exact, off/late path).
  - Biases enter PSUM via tiny diag(b) @ ones matmuls; per-step x
    contributions are small per-step matmuls against a pre-transposed,
    pre-bf16 X tile (no separate projection pipeline).
  - Output: h stored [h_low(part), (t, b, hc)]-friendly layout, PE-transposed
    per 2 steps, PSUM->SBUF f32 copy on GPSIMD, single DMA per 2 timesteps.
"""

import os
import sys
from contextlib import ExitStack

sys.path.insert(0, "/opt/trn_rl_repo")

import numpy as np

_NO_OUT = os.environ.get("GRU_NO_OUT", "0") == "1"      # timing exp only
_NO_XSTAGE = os.environ.get("GRU_NO_XSTAGE", "0") == "1"  # timing exp only

import concourse.bass as bass
import concourse.mybir as mybir
import concourse.tile as tile
from concourse import bacc
from concourse.bass_utils import run_bass_kernel_spmd
from concourse.masks import make_identity

F32 = mybir.dt.float32
BF16 = mybir.dt.bfloat16
AF = mybir.ActivationFunctionType
ALU = mybir.AluOpType

N_CORES = 8
B = 32          # batch per core
S = 512         # sequence length
I = 128         # input features
H = 256         # hidden features
TC = 64         # timesteps per chunk
NCHUNK = S // TC
P = 128

# sigma(x)-0.5 ~= x*(((q3*y + q2)*y + q1)*y + q0), y = x^2 (fit |x|<=5.6)
QS = [0.2402757172521943, -0.014026883800149477, 0.0005286261541401549,
      -7.71991008873346e-06]

_CACHED_NC = None


def _register_sig7():
    """Define + register the fused sigmoid custom DVE op (idempotent)."""
    import concourse.dve_ops as dve_ops
    from concourse.dve_ops import DveOp
    from concourse.dve_spec import (
        C0, C1, C2, C3, Spec, Src0, _has_src1, _spill_c3_to_src1, lower, sq,
    )
    from concourse.dve_uop import DveOpSpec

    for op in dve_ops.OPS:
        if op.name == "ANT_GRU_SIG7":
            return op

    y = sq(Src0)
    body = Src0 * (((C3 * y + C2) * y + C1) * y + C0)

    def ref(in0, in1, s0, s1, imm2):
        yy = in0 * in0
        return (in0 * (((in1 * yy + imm2) * yy + s1) * yy + s0)).astype(
            np.float32
        )

    spec = Spec(body=_spill_c3_to_src1(body), reference=ref)
    uops = lower(spec, ver="v3")
    sha = DveOpSpec(
        name="ANT_GRU_SIG7", opcode=0, uops=uops, rd1_en=_has_src1(spec)
    ).sha("v3")
    op = DveOp("ANT_GRU_SIG7", spec, subdim=False, uops_sha={"v3": sha})
    dve_ops.OPS.append(op)
    dve_ops._SUB_OPCODE_FOR_NAME[op.name] = (
        dve_ops._CUSTOM_DVE_ROW_BASE + len(dve_ops.OPS) - 1
    )
    dve_ops.CUSTOM_DVE_SPECS[op.name] = op.spec
    return op


def _build_nc():
    sig7 = _register_sig7()
    nc = bacc.Bacc(
        "TRN2",
        target_bir_lowering=False,
        debug=False,
        enable_asserts=False,
        num_devices=N_CORES,
    )

    X = nc.dram_tensor("X", [B, S, I], F32, kind="ExternalInput").ap()
    Ws = [
        nc.dram_tensor(n, [H, H + I], F32, kind="ExternalInput").ap()
        for n in ("W_z", "W_r", "W_c")
    ]
    bs = [
        nc.dram_tensor(n, [H], F32, kind="ExternalInput").ap()
        for n in ("b_z", "b_r", "b_c")
    ]
    Y = nc.dram_tensor("Y", [S, B, H], F32, kind="ExternalOutput").ap()

    with tile.TileContext(nc) as tc, ExitStack() as ctx:
        _emit(nc, tc, ctx, sig7, X, Ws, bs, Y)

    nc.compile()
    return nc


def _emit(nc, tc, ctx, sig7, X, Ws, bs, Y):
    const = ctx.enter_context(tc.tile_pool(name="const", bufs=1))
    wtmp_pool = ctx.enter_context(tc.tile_pool(name="wtmp", bufs=2))
    xnpool = ctx.enter_context(tc.tile_pool(name="xn", bufs=2))
    xtpool = ctx.enter_context(tc.tile_pool(name="xt", bufs=2))
    hpool = ctx.enter_context(tc.tile_pool(name="hh", bufs=2))
    rppool = ctx.enter_context(tc.tile_pool(name="rp", bufs=2))
    rhpool = ctx.enter_context(tc.tile_pool(name="rh", bufs=3))
    mupool = ctx.enter_context(tc.tile_pool(name="mu", bufs=3))
    vpool = ctx.enter_context(tc.tile_pool(name="vv", bufs=3))
    zpool = ctx.enter_context(tc.tile_pool(name="zz", bufs=3))
    tpool = ctx.enter_context(tc.tile_pool(name="tt", bufs=3))
    opool = ctx.enter_context(tc.tile_pool(name="ost", bufs=3))
    ppool_t = ctx.enter_context(tc.tile_pool(name="pt", bufs=1, space="PSUM"))
    ppool_r = ctx.enter_context(tc.tile_pool(name="ppr", bufs=2, space="PSUM"))
    ppool_z = ctx.enter_context(tc.tile_pool(name="ppz", bufs=2, space="PSUM"))
    ppool_c = ctx.enter_context(tc.tile_pool(name="ppc", bufs=2, space="PSUM"))

    ident = const.tile([P, P], F32, tag="ident")
    make_identity(nc, ident)
    ident_bf = const.tile([P, P], BF16, tag="identbf")
    nc.scalar.copy(ident_bf, ident)

    # --- weights: lhsT layout [k(part), m] in bf16; negated copies for mu ---
    WT = [[[None] * 3 for _ in range(2)] for _ in range(3)]
    NWT = [[[None] * 2 for _ in range(2)] for _ in range(2)]  # z, r only
    for g in range(3):
        for m in range(2):
            for k in range(3):
                wtmp = wtmp_pool.tile([P, P], F32, tag="wtmp")
                nc.sync.dma_start(
                    wtmp[:], Ws[g][m * P : (m + 1) * P, k * P : (k + 1) * P]
                )
                pt = ppool_t.tile([P, P], F32, tag="pt")
                nc.tensor.transpose(pt, wtmp, ident)
                wl = const.tile([P, P], BF16, tag=f"wl_{g}_{m}_{k}")
                nc.scalar.copy(wl, pt)
                WT[g][m][k] = wl
                if g < 2 and k < 2:
                    nw = const.tile([P, P], BF16, tag=f"nw_{g}_{m}_{k}")
                    nc.vector.tensor_scalar_mul(nw, wl, -1.0)
                    NWT[g][m][k] = nw

    # biases as [128, 2] then diag(b) tiles for the bias matmuls
    diagb = [[None] * 2 for _ in range(3)]
    for g in range(3):
        bt = const.tile([P, 2], F32, tag=f"b_{g}")
        nc.sync.dma_start(bt[:], bs[g].rearrange("(hc p) -> p hc", p=P))
        for m in range(2):
            db = const.tile([P, P], BF16, tag=f"db_{g}_{m}")
            nc.scalar.mul(db, ident, bt[:, m : m + 1])
            diagb[g][m] = db

    ones = const.tile([P, B], BF16, tag="ones")
    nc.vector.memset(ones[:], 1.0)
    q3t = const.tile([P, 1], F32, tag="q3")
    nc.vector.memset(q3t[:], QS[3])
    zero_h = const.tile([P, B, 2], BF16, tag="zh")
    nc.vector.memset(zero_h[:], 0.0)
    zero_v = const.tile([P, B, 2], BF16, tag="zv")
    nc.vector.memset(zero_v[:], 0.0)
    zero_mu = const.tile([P, B, 2], BF16, tag="zmu")
    nc.vector.memset(zero_mu[:], 0.0)

    def emit_x_tile(xt_dst, c, j):
        """Load + transpose X[2j:2j+2, c*TC:(c+1)*TC, :] into xt_dst[:, j]."""
        t0 = c * TC
        xn = xnpool.tile([P, P], F32, tag="xn")
        for boff in range(2):
            nc.sync.dma_start(
                xn[boff * TC : (boff + 1) * TC],
                X[2 * j + boff, t0 : t0 + TC, :],
            )
        pt = ppool_t.tile([P, P], F32, tag="pt")
        nc.tensor.transpose(pt, xn, ident)
        nc.vector.tensor_copy(
            xt_dst[:, j].rearrange("p b t -> p (b t)"), pt
        )

    # chunk 0's x tiles up front; xt layout [p, j, boff, t]
    xt_cur = xtpool.tile([P, 16, 2, TC], BF16, tag="xt")
    for j in range(16):
        emit_x_tile(xt_cur, 0, j)

    h_prev = zero_h
    v_prev = zero_v
    mu_prev = zero_mu

    for c in range(NCHUNK):
        t0 = c * TC
        xt_next = None
        if c + 1 < NCHUNK:
            xt_next = xtpool.tile([P, 16, 2, TC], BF16, tag="xt")
        h_hist = hpool.tile([P, TC, B, 2], BF16, tag="hh")

        for s in range(TC):
            x_rhs = xt_cur[:, :, :, s]  # [P, 16, 2] -> 32 b cols

            pr = ppool_r.tile([P, 2, B], F32, tag="pr")
            pz = ppool_z.tile([P, 2, B], F32, tag="pz")
            pc = ppool_c.tile([P, 2, B], F32, tag="pc")
            # r-gate: contiguous accumulation group per m-half; v-mms last
            # (critical arrival) so sigma_r starts as soon as possible
            for m in range(2):
                nc.tensor.matmul(pr[:, m], lhsT=diagb[1][m], rhs=ones[:],
                                 start=True, stop=False)
                nc.tensor.matmul(pr[:, m], lhsT=WT[1][m][2], rhs=x_rhs,
                                 start=False, stop=False)
                for k in range(2):
                    nc.tensor.matmul(pr[:, m], lhsT=NWT[1][m][k],
                                     rhs=mu_prev[:, :, k],
                                     start=False, stop=False)
                for k in range(2):
                    nc.tensor.matmul(pr[:, m], lhsT=WT[1][m][k],
                                     rhs=v_prev[:, :, k],
                                     start=False, stop=(k == 1))
            # r' on DVE (custom fused sigmoid-0.5)
            rp = rppool.tile([P, 2, B], F32, tag="rp")
            nc.vector._custom_dve(sig7, out=rp[:], in0=pr[:], in1=q3t[:],
                                  s0=QS[0], s1=QS[1], imm2=QS[2])
            # z-gate groups
            for m in range(2):
                nc.tensor.matmul(pz[:, m], lhsT=diagb[0][m], rhs=ones[:],
                                 start=True, stop=False)
                nc.tensor.matmul(pz[:, m], lhsT=WT[0][m][2], rhs=x_rhs,
                                 start=False, stop=False)
                for k in range(2):
                    nc.tensor.matmul(pz[:, m], lhsT=NWT[0][m][k],
                                     rhs=mu_prev[:, :, k],
                                     start=False, stop=False)
                for k in range(2):
                    nc.tensor.matmul(pz[:, m], lhsT=WT[0][m][k],
                                     rhs=v_prev[:, :, k],
                                     start=False, stop=(k == 1))
            # z on Act
            z_s = zpool.tile([P, B, 2], BF16, tag="z")
            nc.scalar.activation(z_s[:], pz.rearrange("p m b -> p b m"),
                                 AF.Sigmoid)
            # rh = (r' + 0.5) * h_prev
            rh = rhpool.tile([P, B, 2], BF16, tag="rh")
            nc.vector.scalar_tensor_tensor(
                rh[:], rp.rearrange("p m b -> p b m"), 0.5, h_prev[:],
                ALU.add, ALU.mult,
            )
            # candidate groups
            for m in range(2):
                nc.tensor.matmul(pc[:, m], lhsT=diagb[2][m], rhs=ones[:],
                                 start=True, stop=False)
                nc.tensor.matmul(pc[:, m], lhsT=WT[2][m][2], rhs=x_rhs,
                                 start=False, stop=False)
                for k in range(2):
                    nc.tensor.matmul(pc[:, m], lhsT=WT[2][m][k],
                                     rhs=rh[:, :, k],
                                     start=False, stop=(k == 1))
            # mu_s = (z - 1) * h_prev
            mu_s = mupool.tile([P, B, 2], BF16, tag="mu")
            nc.vector.scalar_tensor_tensor(
                mu_s[:], z_s[:], 1.0, h_prev[:], ALU.subtract, ALU.mult,
            )
            # tanh on Act
            T_s = tpool.tile([P, B, 2], BF16, tag="T")
            nc.scalar.activation(T_s[:], pc.rearrange("p m b -> p b m"),
                                 AF.Tanh)
            # v_s = z * T ; h_s = v - mu
            v_s = vpool.tile([P, B, 2], BF16, tag="v")
            nc.vector.tensor_mul(v_s[:], z_s[:], T_s[:])
            nc.vector.tensor_sub(h_hist[:, s], v_s[:], mu_s[:])

            h_prev = h_hist[:, s]
            v_prev = v_s
            mu_prev = mu_s

            # output transpose + DMA every 2 steps
            if s % 2 == 1 and not _NO_OUT:
                ptb = ppool_t.tile([P, P], BF16, tag="ptb")
                nc.tensor.transpose(
                    ptb,
                    h_hist[:, s - 1 : s + 1].rearrange(
                        "p t b hc -> p (t b hc)"
                    ),
                    ident_bf,
                )
                ost = opool.tile([P, P], F32, tag="ost")
                nc.scalar.copy(ost[:], ptb)
                nc.sync.dma_start(
                    Y[t0 + s - 1 : t0 + s + 1, :, :].rearrange(
                        "t b (hc hl) -> (t b hc) hl", hc=2
                    ),
                    ost[:],
                )

            # stage next chunk's x tiles (1 per 4 steps)
            if xt_next is not None and s % 4 == 0 and not _NO_XSTAGE:
                emit_x_tile(xt_next, c + 1, s // 4)

        if xt_next is not None and _NO_XSTAGE:
            for j in range(16):
                emit_x_tile(xt_next, c + 1, j)
        xt_cur = xt_next


def _get_nc():
    global _CACHED_NC
    if _CACHED_NC is None:
        _CACHED_NC = _build_nc()
    return _CACHED_NC


def _run(inputs, trace=False):
    nc = _get_nc()
    X = np.ascontiguousarray(np.asarray(inputs["X"], dtype=np.float32))
    names = ("W_z", "b_z", "W_r", "b_r", "W_c", "b_c")
    shared = {
        n: np.ascontiguousarray(np.asarray(inputs[n], dtype=np.float32))
        for n in names
    }
    in_maps = []
    for core in range(N_CORES):
        m = {"X": np.ascontiguousarray(X[core * B : (core + 1) * B])}
        m.update(shared)
        in_maps.append(m)
    res = run_bass_kernel_spmd(nc, in_maps, list(range(N_CORES)), trace=trace)
    out = np.concatenate([res.results[c]["Y"] for c in range(N_CORES)], axis=1)
    return out, res


def kernel(**inputs) -> np.ndarray:
    out, _ = _run(inputs, trace=False)
    return out


# revision 19
# speedup vs baseline: 1.0211x; 1.0211x over previous
"""GRU kernel for Trainium2 (8 NeuronCores, data-parallel over batch).

Problem: nn_GRU — X [256, 512, 128] f32, W_z/W_r/W_c [256, 384], b_* [256].
Output: h_history [512, 256, 256] f32.

Sharding: batch 256 -> 8 cores x 32. Each core runs an independent GRU
recurrence over its batch shard; weights replicated; no collectives.

Design (latency-oriented: the 512-step recurrence is serial):
  - bf16 matmul operands, fp32 PSUM accumulation.
  - h_t is carried as the pair (v_t, mu_t) with v = z*c, mu = (z-1)*h_prev,
    h = v - mu. The recurrence matmuls consume v and mu directly (mu through
    negated weight copies), so the h-combine leaves the critical path.
  - r-gate sigmoid is a single fused custom DVE op (deg-7 odd minimax of
    sigma-0.5; r preacts stay within its fit range), followed by one
    scalar_tensor_tensor for rh = (r'+0.5)*h. The Activation engine only
    handles the z-sigmoid and candidate tanh (exact, off/late path).
  - Biases enter PSUM via tiny diag(b) @ ones matmuls; per-step x
    contributions are small per-step matmuls against a pre-transposed,
    pre-bf16 X tile (no separate projection pipeline).
  - Output: h stored [h_low(part), (t, b, hc)]-friendly layout, PE-transposed
    per 2 steps, PSUM->SBUF f32 copy on GPSIMD, single DMA per 2 timesteps.
"""

import os
import sys
from contextlib import ExitStack

sys.path.insert(0, "/opt/trn_rl_repo")

import numpy as np

_NO_OUT = os.environ.get("GRU_NO_OUT", "0") == "1"      # timing exp only
_NO_XSTAGE = os.environ.get("GRU_NO_XSTAGE", "0") == "1"  # timing exp only
_ABL = os.environ.get("GRU_ABL", "")  # comma list: sigz,tanh,r,mu,hcomb

import concourse.bass as bass
import concourse.mybir as mybir
import concourse.tile as tile
from concourse import bacc
from concourse.bass_utils import run_bass_kernel_spmd
from concourse.masks import make_identity

F32 = mybir.dt.float32
BF16 = mybir.dt.bfloat16
AF = mybir.ActivationFunctionType
ALU = mybir.AluOpType

N_CORES = 8
B = 32          # batch per core
S = 512         # sequence length
I = 128         # input features
H = 256         # hidden features
TC = 64         # timesteps per chunk
NCHUNK = S // TC
P = 128

# sigma(x)-0.5 ~= x*(((q3*y + q2)*y + q1)*y + q0), y = x^2 (fit |x|<=5.6)
QS = [0.2402757172521943, -0.014026883800149477, 0.0005286261541401549,
      -7.71991008873346e-06]

_CACHED_NC = None


def _register_sig7():
    """Define + register the fused sigmoid custom DVE op (idempotent)."""
    import concourse.dve_ops as dve_ops
    from concourse.dve_ops import DveOp
    from concourse.dve_spec import (
        C0, C1, C2, C3, Spec, Src0, _has_src1, _spill_c3_to_src1, lower, sq,
    )
    from concourse.dve_uop import DveOpSpec

    for op in dve_ops.OPS:
        if op.name == "ANT_GRU_SIG7":
            return op

    y = sq(Src0)
    body = Src0 * (((C3 * y + C2) * y + C1) * y + C0)

    def ref(in0, in1, s0, s1, imm2):
        yy = in0 * in0
        return (in0 * (((in1 * yy + imm2) * yy + s1) * yy + s0)).astype(
            np.float32
        )

    spec = Spec(body=_spill_c3_to_src1(body), reference=ref)
    uops = lower(spec, ver="v3")
    sha = DveOpSpec(
        name="ANT_GRU_SIG7", opcode=0, uops=uops, rd1_en=_has_src1(spec)
    ).sha("v3")
    op = DveOp("ANT_GRU_SIG7", spec, subdim=False, uops_sha={"v3": sha})
    dve_ops.OPS.append(op)
    dve_ops._SUB_OPCODE_FOR_NAME[op.name] = (
        dve_ops._CUSTOM_DVE_ROW_BASE + len(dve_ops.OPS) - 1
    )
    dve_ops.CUSTOM_DVE_SPECS[op.name] = op.spec
    return op


def _build_nc():
    sig7 = _register_sig7()
    nc = bacc.Bacc(
        "TRN2",
        target_bir_lowering=False,
        debug=False,
        enable_asserts=False,
        num_devices=N_CORES,
    )

    X = nc.dram_tensor("X", [B, S, I], F32, kind="ExternalInput").ap()
    Ws = [
        nc.dram_tensor(n, [H, H + I], F32, kind="ExternalInput").ap()
        for n in ("W_z", "W_r", "W_c")
    ]
    bs = [
        nc.dram_tensor(n, [H], F32, kind="ExternalInput").ap()
        for n in ("b_z", "b_r", "b_c")
    ]
    Y = nc.dram_tensor("Y", [S, B, H], F32, kind="ExternalOutput").ap()

    with tile.TileContext(nc) as tc, ExitStack() as ctx:
        _emit(nc, tc, ctx, sig7, X, Ws, bs, Y)

    nc.compile()
    return nc


def _emit(nc, tc, ctx, sig7, X, Ws, bs, Y):
    const = ctx.enter_context(tc.tile_pool(name="const", bufs=1))
    wtmp_pool = ctx.enter_context(tc.tile_pool(name="wtmp", bufs=2))
    xnpool = ctx.enter_context(tc.tile_pool(name="xn", bufs=2))
    xtpool = ctx.enter_context(tc.tile_pool(name="xt", bufs=2))
    hpool = ctx.enter_context(tc.tile_pool(name="hh", bufs=2))
    rppool = ctx.enter_context(tc.tile_pool(name="rp", bufs=2))
    rhpool = ctx.enter_context(tc.tile_pool(name="rh", bufs=3))
    mupool = ctx.enter_context(tc.tile_pool(name="mu", bufs=3))
    vpool = ctx.enter_context(tc.tile_pool(name="vv", bufs=3))
    zpool = ctx.enter_context(tc.tile_pool(name="zz", bufs=3))
    tpool = ctx.enter_context(tc.tile_pool(name="tt", bufs=3))
    opool = ctx.enter_context(tc.tile_pool(name="ost", bufs=3))
    ppool_t = ctx.enter_context(tc.tile_pool(name="pt", bufs=1, space="PSUM"))
    ppool_r = ctx.enter_context(tc.tile_pool(name="ppr", bufs=2, space="PSUM"))
    ppool_z = ctx.enter_context(tc.tile_pool(name="ppz", bufs=2, space="PSUM"))
    ppool_c = ctx.enter_context(tc.tile_pool(name="ppc", bufs=2, space="PSUM"))

    ident = const.tile([P, P], F32, tag="ident")
    make_identity(nc, ident)
    ident_bf = const.tile([P, P], BF16, tag="identbf")
    nc.scalar.copy(ident_bf, ident)

    # --- weights: lhsT layout [k(part), m] in bf16; negated copies for mu ---
    WT = [[[None] * 3 for _ in range(2)] for _ in range(3)]
    NWT = [[[None] * 2 for _ in range(2)] for _ in range(2)]  # z, r only
    for g in range(3):
        for m in range(2):
            for k in range(3):
                wtmp = wtmp_pool.tile([P, P], F32, tag="wtmp")
                nc.sync.dma_start(
                    wtmp[:], Ws[g][m * P : (m + 1) * P, k * P : (k + 1) * P]
                )
                pt = ppool_t.tile([P, P], F32, tag="pt")
                nc.tensor.transpose(pt, wtmp, ident)
                wl = const.tile([P, P], BF16, tag=f"wl_{g}_{m}_{k}")
                nc.scalar.copy(wl, pt)
                WT[g][m][k] = wl
                if g < 2 and k < 2:
                    nw = const.tile([P, P], BF16, tag=f"nw_{g}_{m}_{k}")
                    nc.vector.tensor_scalar_mul(nw, wl, -1.0)
                    NWT[g][m][k] = nw

    # biases as [128, 2] then diag(b) tiles for the bias matmuls
    diagb = [[None] * 2 for _ in range(3)]
    for g in range(3):
        bt = const.tile([P, 2], F32, tag=f"b_{g}")
        nc.sync.dma_start(bt[:], bs[g].rearrange("(hc p) -> p hc", p=P))
        for m in range(2):
            db = const.tile([P, P], BF16, tag=f"db_{g}_{m}")
            nc.scalar.mul(db, ident, bt[:, m : m + 1])
            diagb[g][m] = db

    ones = const.tile([P, B], BF16, tag="ones")
    nc.vector.memset(ones[:], 1.0)
    q3t = const.tile([P, 1], F32, tag="q3")
    nc.vector.memset(q3t[:], QS[3])
    zero_h = const.tile([P, B, 2], BF16, tag="zh")
    nc.vector.memset(zero_h[:], 0.0)
    zero_v = const.tile([P, B, 2], BF16, tag="zv")
    nc.vector.memset(zero_v[:], 0.0)
    zero_mu = const.tile([P, B, 2], BF16, tag="zmu")
    nc.vector.memset(zero_mu[:], 0.0)

    def emit_x_tile(xt_dst, c, j):
        """Load + transpose X[2j:2j+2, c*TC:(c+1)*TC, :] into xt_dst[:, j]."""
        t0 = c * TC
        xn = xnpool.tile([P, P], F32, tag="xn")
        for boff in range(2):
            nc.sync.dma_start(
                xn[boff * TC : (boff + 1) * TC],
                X[2 * j + boff, t0 : t0 + TC, :],
            )
        pt = ppool_t.tile([P, P], F32, tag="pt")
        nc.tensor.transpose(pt, xn, ident)
        nc.vector.tensor_copy(
            xt_dst[:, j].rearrange("p b t -> p (b t)"), pt
        )

    # chunk 0's x tiles up front; xt layout [p, j, boff, t]
    xt_cur = xtpool.tile([P, 16, 2, TC], BF16, tag="xt")
    for j in range(16):
        emit_x_tile(xt_cur, 0, j)

    h_prev = zero_h
    v_prev = zero_v
    mu_prev = zero_mu

    for c in range(NCHUNK):
        t0 = c * TC
        xt_next = None
        if c + 1 < NCHUNK:
            xt_next = xtpool.tile([P, 16, 2, TC], BF16, tag="xt")
        h_hist = hpool.tile([P, TC, B, 2], BF16, tag="hh")

        for s in range(TC):
            x_rhs = xt_cur[:, :, :, s]  # [P, 16, 2] -> 32 b cols

            pr = ppool_r.tile([P, 2, B], F32, tag="pr")
            pz = ppool_z.tile([P, 2, B], F32, tag="pz")
            pc = ppool_c.tile([P, 2, B], F32, tag="pc")
            # r-gate: contiguous accumulation group per m-half; v-mms last
            # (critical arrival) so sigma_r starts as soon as possible
            for m in range(2):
                nc.tensor.matmul(pr[:, m], lhsT=diagb[1][m], rhs=ones[:],
                                 start=True, stop=False)
                nc.tensor.matmul(pr[:, m], lhsT=WT[1][m][2], rhs=x_rhs,
                                 start=False, stop=False)
                for k in range(2):
                    nc.tensor.matmul(pr[:, m], lhsT=NWT[1][m][k],
                                     rhs=mu_prev[:, :, k],
                                     start=False, stop=False)
                for k in range(2):
                    nc.tensor.matmul(pr[:, m], lhsT=WT[1][m][k],
                                     rhs=v_prev[:, :, k],
                                     start=False, stop=(k == 1))
            # r' on DVE (custom fused sigmoid-0.5)
            rp = rppool.tile([P, 2, B], F32, tag="rp")
            nc.vector._custom_dve(sig7, out=rp[:], in0=pr[:], in1=q3t[:],
                                  s0=QS[0], s1=QS[1], imm2=QS[2])
            # z-gate groups
            for m in range(2):
                nc.tensor.matmul(pz[:, m], lhsT=diagb[0][m], rhs=ones[:],
                                 start=True, stop=False)
                nc.tensor.matmul(pz[:, m], lhsT=WT[0][m][2], rhs=x_rhs,
                                 start=False, stop=False)
                for k in range(2):
                    nc.tensor.matmul(pz[:, m], lhsT=NWT[0][m][k],
                                     rhs=mu_prev[:, :, k],
                                     start=False, stop=False)
                for k in range(2):
                    nc.tensor.matmul(pz[:, m], lhsT=WT[0][m][k],
                                     rhs=v_prev[:, :, k],
                                     start=False, stop=(k == 1))
            # z on Act
            if "sigz" in _ABL:
                z_s = zero_h
            else:
                z_s = zpool.tile([P, B, 2], BF16, tag="z")
                nc.scalar.activation(z_s[:], pz.rearrange("p m b -> p b m"),
                                     AF.Sigmoid)
            # rh = (r' + 0.5) * h_prev
            if "r" in _ABL.split(","):
                rh = h_prev
            else:
                rh = rhpool.tile([P, B, 2], BF16, tag="rh")
                nc.vector.scalar_tensor_tensor(
                    rh[:], rp.rearrange("p m b -> p b m"), 0.5, h_prev[:],
                    ALU.add, ALU.mult,
                )
            # candidate groups
            for m in range(2):
                nc.tensor.matmul(pc[:, m], lhsT=diagb[2][m], rhs=ones[:],
                                 start=True, stop=False)
                nc.tensor.matmul(pc[:, m], lhsT=WT[2][m][2], rhs=x_rhs,
                                 start=False, stop=False)
                for k in range(2):
                    nc.tensor.matmul(pc[:, m], lhsT=WT[2][m][k],
                                     rhs=rh[:, :, k],
                                     start=False, stop=(k == 1))
            # mu_s = (z - 1) * h_prev
            if "mu" in _ABL:
                mu_s = zero_mu
            else:
                mu_s = mupool.tile([P, B, 2], BF16, tag="mu")
                nc.vector.scalar_tensor_tensor(
                    mu_s[:], z_s[:], 1.0, h_prev[:], ALU.subtract, ALU.mult,
                )
            # tanh on Act
            if "tanh" in _ABL:
                T_s = z_s
            else:
                T_s = tpool.tile([P, B, 2], BF16, tag="T")
                nc.scalar.activation(T_s[:], pc.rearrange("p m b -> p b m"),
                                     AF.Tanh)
            # v_s = z * T ; h_s = v - mu
            v_s = vpool.tile([P, B, 2], BF16, tag="v")
            nc.vector.tensor_mul(v_s[:], z_s[:], T_s[:])
            nc.vector.tensor_sub(h_hist[:, s], v_s[:], mu_s[:])

            h_prev = h_hist[:, s]
            v_prev = v_s
            mu_prev = mu_s

            # output transpose + DMA every 2 steps
            if s % 2 == 1 and not _NO_OUT:
                ptb = ppool_t.tile([P, P], BF16, tag="ptb")
                nc.tensor.transpose(
                    ptb,
                    h_hist[:, s - 1 : s + 1].rearrange(
                        "p t b hc -> p (t b hc)"
                    ),
                    ident_bf,
                )
                ost = opool.tile([P, P], F32, tag="ost")
                nc.scalar.copy(ost[:], ptb)
                nc.sync.dma_start(
                    Y[t0 + s - 1 : t0 + s + 1, :, :].rearrange(
                        "t b (hc hl) -> (t b hc) hl", hc=2
                    ),
                    ost[:],
                )

            # stage next chunk's x tiles (1 per 4 steps)
            if xt_next is not None and s % 4 == 0 and not _NO_XSTAGE:
                emit_x_tile(xt_next, c + 1, s // 4)

        if xt_next is not None and _NO_XSTAGE:
            for j in range(16):
                emit_x_tile(xt_next, c + 1, j)
        xt_cur = xt_next


def _get_nc():
    global _CACHED_NC
    if _CACHED_NC is None:
        _CACHED_NC = _build_nc()
    return _CACHED_NC


def _run(inputs, trace=False):
    nc = _get_nc()
    X = np.ascontiguousarray(np.asarray(inputs["X"], dtype=np.float32))
    names = ("W_z", "b_z", "W_r", "b_r", "W_c", "b_c")
    shared = {
        n: np.ascontiguousarray(np.asarray(inputs[n], dtype=np.float32))
        for n in names
    }
    in_maps = []
    for core in range(N_CORES):
        m = {"X": np.ascontiguousarray(X[core * B : (core + 1) * B])}
        m.update(shared)
        in_maps.append(m)
    res = run_bass_kernel_spmd(nc, in_maps, list(range(N_CORES)), trace=trace)
    out = np.concatenate([res.results[c]["Y"] for c in range(N_CORES)], axis=1)
    return out, res


def kernel(**inputs) -> np.ndarray:
    out, _ = _run(inputs, trace=False)
    return out


# revision 26
# speedup vs baseline: 1.0818x; 1.0595x over previous
"""GRU kernel for Trainium2 (8 NeuronCores, data-parallel over batch).

Problem: nn_GRU — X [256, 512, 128] f32, W_z/W_r/W_c [256, 384], b_* [256].
Output: h_history [512, 256, 256] f32.

Sharding: batch 256 -> 8 cores x 32. Each core runs an independent GRU
recurrence over its batch shard; weights replicated; no collectives.

Design (latency-oriented: the 512-step recurrence is serial):
  - bf16 matmul operands, fp32 PSUM accumulation.
  - h_t is carried as the pair (v_t, mu_t) with v = z*c, mu = (z-1)*h_prev,
    h = v - mu. The recurrence matmuls consume v and mu directly (mu through
    negated weight copies), so the h-combine leaves the critical path.
  - r-gate sigmoid is a single fused custom DVE op (deg-7 odd minimax of
    sigma-0.5; r preacts stay within its fit range), followed by one
    scalar_tensor_tensor for rh = (r'+0.5)*h. The Activation engine only
    handles the z-sigmoid and candidate tanh (exact, off/late path).
  - Biases enter PSUM via tiny diag(b) @ ones matmuls; per-step x
    contributions are small per-step matmuls against a pre-transposed,
    pre-bf16 X tile (no separate projection pipeline).
  - Output: h stored [h_low(part), (t, b, hc)]-friendly layout, PE-transposed
    per 2 steps, PSUM->SBUF f32 copy on GPSIMD, single DMA per 2 timesteps.
"""

import os
import sys
from contextlib import ExitStack

sys.path.insert(0, "/opt/trn_rl_repo")

import numpy as np

_NO_OUT = os.environ.get("GRU_NO_OUT", "0") == "1"      # timing exp only
_NO_XSTAGE = os.environ.get("GRU_NO_XSTAGE", "0") == "1"  # timing exp only
_ABL = os.environ.get("GRU_ABL", "")  # comma list: sigz,tanh,r,mu,hcomb

import concourse.bass as bass
import concourse.mybir as mybir
import concourse.tile as tile
from concourse import bacc
from concourse.bass_utils import run_bass_kernel_spmd
from concourse.masks import make_identity

F32 = mybir.dt.float32
BF16 = mybir.dt.bfloat16
AF = mybir.ActivationFunctionType
ALU = mybir.AluOpType

N_CORES = 8
B = 32          # batch per core
S = 512         # sequence length
I = 128         # input features
H = 256         # hidden features
TC = 64         # timesteps per chunk
NCHUNK = S // TC
P = 128

# sigma(x)-0.5 ~= x*(((q3*y + q2)*y + q1)*y + q0), y = x^2 (fit |x|<=5.6)
QS = [0.2402757172521943, -0.014026883800149477, 0.0005286261541401549,
      -7.71991008873346e-06]

_CACHED_NC = None


def _register_sig7():
    """Define + register the fused sigmoid custom DVE op (idempotent)."""
    import concourse.dve_ops as dve_ops
    from concourse.dve_ops import DveOp
    from concourse.dve_spec import (
        C0, C1, C2, C3, Spec, Src0, _has_src1, _spill_c3_to_src1, lower, sq,
    )
    from concourse.dve_uop import DveOpSpec

    for op in dve_ops.OPS:
        if op.name == "ANT_GRU_SIG7":
            return op

    y = sq(Src0)
    body = Src0 * (((C3 * y + C2) * y + C1) * y + C0)

    def ref(in0, in1, s0, s1, imm2):
        yy = in0 * in0
        return (in0 * (((in1 * yy + imm2) * yy + s1) * yy + s0)).astype(
            np.float32
        )

    spec = Spec(body=_spill_c3_to_src1(body), reference=ref)
    uops = lower(spec, ver="v3")
    sha = DveOpSpec(
        name="ANT_GRU_SIG7", opcode=0, uops=uops, rd1_en=_has_src1(spec)
    ).sha("v3")
    op = DveOp("ANT_GRU_SIG7", spec, subdim=False, uops_sha={"v3": sha})
    dve_ops.OPS.append(op)
    dve_ops._SUB_OPCODE_FOR_NAME[op.name] = (
        dve_ops._CUSTOM_DVE_ROW_BASE + len(dve_ops.OPS) - 1
    )
    dve_ops.CUSTOM_DVE_SPECS[op.name] = op.spec
    return op


def _build_nc():
    sig7 = _register_sig7()
    nc = bacc.Bacc(
        "TRN2",
        target_bir_lowering=False,
        debug=False,
        enable_asserts=False,
        num_devices=N_CORES,
    )

    X = nc.dram_tensor("X", [B, S, I], F32, kind="ExternalInput").ap()
    Ws = [
        nc.dram_tensor(n, [H, H + I], F32, kind="ExternalInput").ap()
        for n in ("W_z", "W_r", "W_c")
    ]
    bs = [
        nc.dram_tensor(n, [H], F32, kind="ExternalInput").ap()
        for n in ("b_z", "b_r", "b_c")
    ]
    Y = nc.dram_tensor("Y", [S, B, H], F32, kind="ExternalOutput").ap()

    with tile.TileContext(nc) as tc, ExitStack() as ctx:
        _emit(nc, tc, ctx, sig7, X, Ws, bs, Y)

    nc.compile()
    return nc


def _emit(nc, tc, ctx, sig7, X, Ws, bs, Y):
    const = ctx.enter_context(tc.tile_pool(name="const", bufs=1))
    wtmp_pool = ctx.enter_context(tc.tile_pool(name="wtmp", bufs=2))
    xnpool = ctx.enter_context(tc.tile_pool(name="xn", bufs=2))
    xtpool = ctx.enter_context(tc.tile_pool(name="xt", bufs=2))
    hpool = ctx.enter_context(tc.tile_pool(name="hh", bufs=2))
    rppool = ctx.enter_context(tc.tile_pool(name="rp", bufs=2))
    rhpool = ctx.enter_context(tc.tile_pool(name="rh", bufs=2))
    mupool = ctx.enter_context(tc.tile_pool(name="mu", bufs=2))
    vpool = ctx.enter_context(tc.tile_pool(name="vv", bufs=2))
    zpool = ctx.enter_context(tc.tile_pool(name="zz", bufs=2))
    tpool = ctx.enter_context(tc.tile_pool(name="tt", bufs=2))
    opool = ctx.enter_context(tc.tile_pool(name="ost", bufs=2))
    ppool_t = ctx.enter_context(tc.tile_pool(name="pt", bufs=1, space="PSUM"))
    ppool_r = ctx.enter_context(tc.tile_pool(name="ppr", bufs=2, space="PSUM"))
    ppool_z = ctx.enter_context(tc.tile_pool(name="ppz", bufs=2, space="PSUM"))
    ppool_c = ctx.enter_context(tc.tile_pool(name="ppc", bufs=2, space="PSUM"))

    ident = const.tile([P, P], F32, tag="ident")
    make_identity(nc, ident)
    ident_bf = const.tile([P, P], BF16, tag="identbf")
    nc.scalar.copy(ident_bf, ident)

    # --- weights: lhsT layout [k(part), m] in bf16; negated copies for mu ---
    WT = [[[None] * 3 for _ in range(2)] for _ in range(3)]
    NWT = [[[None] * 2 for _ in range(2)] for _ in range(2)]  # z, r only
    for g in range(3):
        for m in range(2):
            for k in range(3):
                wtmp = wtmp_pool.tile([P, P], F32, tag="wtmp")
                nc.sync.dma_start(
                    wtmp[:], Ws[g][m * P : (m + 1) * P, k * P : (k + 1) * P]
                )
                pt = ppool_t.tile([P, P], F32, tag="pt")
                nc.tensor.transpose(pt, wtmp, ident)
                wl = const.tile([P, P], BF16, tag=f"wl_{g}_{m}_{k}")
                nc.scalar.copy(wl, pt)
                WT[g][m][k] = wl
                if g < 2 and k < 2:
                    nw = const.tile([P, P], BF16, tag=f"nw_{g}_{m}_{k}")
                    nc.vector.tensor_scalar_mul(nw, wl, -1.0)
                    NWT[g][m][k] = nw

    # biases as [128, 2] then diag(b) tiles for the bias matmuls
    diagb = [[None] * 2 for _ in range(3)]
    for g in range(3):
        bt = const.tile([P, 2], F32, tag=f"b_{g}")
        nc.sync.dma_start(bt[:], bs[g].rearrange("(hc p) -> p hc", p=P))
        for m in range(2):
            db = const.tile([P, P], BF16, tag=f"db_{g}_{m}")
            nc.scalar.mul(db, ident, bt[:, m : m + 1])
            diagb[g][m] = db

    ones = const.tile([P, B], BF16, tag="ones")
    nc.vector.memset(ones[:], 1.0)
    q3t = const.tile([P, 1], F32, tag="q3")
    nc.vector.memset(q3t[:], QS[3])
    zero_h = const.tile([P, B, 2], BF16, tag="zh")
    nc.vector.memset(zero_h[:], 0.0)
    zero_v = const.tile([P, B, 2], BF16, tag="zv")
    nc.vector.memset(zero_v[:], 0.0)
    zero_mu = const.tile([P, B, 2], BF16, tag="zmu")
    nc.vector.memset(zero_mu[:], 0.0)

    def emit_x_tile(xt_dst, c, j):
        """Load + transpose X[2j:2j+2, c*TC:(c+1)*TC, :] into xt_dst[:, j]."""
        t0 = c * TC
        xn = xnpool.tile([P, P], F32, tag="xn")
        for boff in range(2):
            nc.sync.dma_start(
                xn[boff * TC : (boff + 1) * TC],
                X[2 * j + boff, t0 : t0 + TC, :],
            )
        pt = ppool_t.tile([P, P], F32, tag="pt")
        nc.tensor.transpose(pt, xn, ident)
        nc.vector.tensor_copy(
            xt_dst[:, j].rearrange("p b t -> p (b t)"), pt
        )

    # chunk 0's x tiles up front; xt layout [p, j, boff, t]
    xt_cur = xtpool.tile([P, 16, 2, TC], BF16, tag="xt")
    for j in range(16):
        emit_x_tile(xt_cur, 0, j)

    h_prev = zero_h[:]
    v_prev = zero_v[:]
    mu_prev = zero_mu[:]

    for c in range(NCHUNK):
        t0 = c * TC
        xt_next = None
        if c + 1 < NCHUNK:
            xt_next = xtpool.tile([P, 16, 2, TC], BF16, tag="xt")
        h_hist = hpool.tile([P, TC, B, 2], BF16, tag="hh")
        # per-chunk rings: fresh slice per step -> no per-step WAR sems
        rp_ring = rppool.tile([P, TC, 2, B], F32, tag="rp")
        rh_ring = rhpool.tile([P, TC, B, 2], BF16, tag="rh")
        mu_ring = mupool.tile([P, TC, B, 2], BF16, tag="mu")
        v_ring = vpool.tile([P, TC, B, 2], BF16, tag="vv")
        z_ring = zpool.tile([P, TC, B, 2], BF16, tag="zz")
        T_ring = tpool.tile([P, TC, B, 2], BF16, tag="tt")
        ost_ring = opool.tile([P, TC // 2, P], F32, tag="ost")

        for s in range(TC):
            x_rhs = xt_cur[:, :, :, s]  # [P, 16, 2] -> 32 b cols

            pr = ppool_r.tile([P, 2, B], F32, tag="pr")
            pz = ppool_z.tile([P, 2, B], F32, tag="pz")
            pc = ppool_c.tile([P, 2, B], F32, tag="pc")
            # r-gate: contiguous accumulation group per m-half; v-mms last
            # (critical arrival) so sigma_r starts as soon as possible
            for m in range(2):
                nc.tensor.matmul(pr[:, m], lhsT=diagb[1][m], rhs=ones[:],
                                 start=True, stop=False)
                nc.tensor.matmul(pr[:, m], lhsT=WT[1][m][2], rhs=x_rhs,
                                 start=False, stop=False)
                for k in range(2):
                    nc.tensor.matmul(pr[:, m], lhsT=NWT[1][m][k],
                                     rhs=mu_prev[:, :, k],
                                     start=False, stop=False)
                for k in range(2):
                    nc.tensor.matmul(pr[:, m], lhsT=WT[1][m][k],
                                     rhs=v_prev[:, :, k],
                                     start=False, stop=(k == 1))
            # r' on DVE (custom fused sigmoid-0.5)
            rp = rp_ring[:, s]
            nc.vector._custom_dve(sig7, out=rp, in0=pr[:], in1=q3t[:],
                                  s0=QS[0], s1=QS[1], imm2=QS[2])
            # z-gate groups
            for m in range(2):
                nc.tensor.matmul(pz[:, m], lhsT=diagb[0][m], rhs=ones[:],
                                 start=True, stop=False)
                nc.tensor.matmul(pz[:, m], lhsT=WT[0][m][2], rhs=x_rhs,
                                 start=False, stop=False)
                for k in range(2):
                    nc.tensor.matmul(pz[:, m], lhsT=NWT[0][m][k],
                                     rhs=mu_prev[:, :, k],
                                     start=False, stop=False)
                for k in range(2):
                    nc.tensor.matmul(pz[:, m], lhsT=WT[0][m][k],
                                     rhs=v_prev[:, :, k],
                                     start=False, stop=(k == 1))
            # z on Act
            if "sigz" in _ABL:
                z_s = zero_h[:]
            else:
                z_s = z_ring[:, s]
                nc.scalar.activation(z_s, pz.rearrange("p m b -> p b m"),
                                     AF.Sigmoid)
            # rh = (r' + 0.5) * h_prev
            if "r" in _ABL.split(","):
                rh = h_prev
            else:
                rh = rh_ring[:, s]
                nc.vector.scalar_tensor_tensor(
                    rh, rp.rearrange("p m b -> p b m"), 0.5, h_prev,
                    ALU.add, ALU.mult,
                )
            # candidate groups
            for m in range(2):
                nc.tensor.matmul(pc[:, m], lhsT=diagb[2][m], rhs=ones[:],
                                 start=True, stop=False)
                nc.tensor.matmul(pc[:, m], lhsT=WT[2][m][2], rhs=x_rhs,
                                 start=False, stop=False)
                for k in range(2):
                    nc.tensor.matmul(pc[:, m], lhsT=WT[2][m][k],
                                     rhs=rh[:, :, k],
                                     start=False, stop=(k == 1))
            # mu_s = (z - 1) * h_prev
            if "mu" in _ABL:
                mu_s = zero_mu[:]
            else:
                mu_s = mu_ring[:, s]
                nc.vector.scalar_tensor_tensor(
                    mu_s, z_s, 1.0, h_prev, ALU.subtract, ALU.mult,
                )
            # tanh on Act
            if "tanh" in _ABL:
                T_s = z_s
            else:
                T_s = T_ring[:, s]
                nc.scalar.activation(T_s, pc.rearrange("p m b -> p b m"),
                                     AF.Tanh)
            # v_s = z * T ; h_s = v - mu
            v_s = v_ring[:, s]
            nc.vector.tensor_mul(v_s, z_s, T_s)
            nc.vector.tensor_sub(h_hist[:, s], v_s, mu_s)

            h_prev = h_hist[:, s]
            v_prev = v_s
            mu_prev = mu_s

            # output transpose + DMA every 2 steps
            if s % 2 == 1 and not _NO_OUT:
                ptb = ppool_t.tile([P, P], BF16, tag="ptb")
                nc.tensor.transpose(
                    ptb,
                    h_hist[:, s - 1 : s + 1].rearrange(
                        "p t b hc -> p (t b hc)"
                    ),
                    ident_bf,
                )
                ost = ost_ring[:, s // 2]
                nc.scalar.copy(ost, ptb)
                nc.sync.dma_start(
                    Y[t0 + s - 1 : t0 + s + 1, :, :].rearrange(
                        "t b (hc hl) -> (t b hc) hl", hc=2
                    ),
                    ost,
                )

            # stage next chunk's x tiles (1 per 4 steps)
            if xt_next is not None and s % 4 == 0 and not _NO_XSTAGE:
                emit_x_tile(xt_next, c + 1, s // 4)

        if xt_next is not None and _NO_XSTAGE:
            for j in range(16):
                emit_x_tile(xt_next, c + 1, j)
        xt_cur = xt_next


def _get_nc():
    global _CACHED_NC
    if _CACHED_NC is None:
        _CACHED_NC = _build_nc()
    return _CACHED_NC


def _run(inputs, trace=False):
    nc = _get_nc()
    X = np.ascontiguousarray(np.asarray(inputs["X"], dtype=np.float32))
    names = ("W_z", "b_z", "W_r", "b_r", "W_c", "b_c")
    shared = {
        n: np.ascontiguousarray(np.asarray(inputs[n], dtype=np.float32))
        for n in names
    }
    in_maps = []
    for core in range(N_CORES):
        m = {"X": np.ascontiguousarray(X[core * B : (core + 1) * B])}
        m.update(shared)
        in_maps.append(m)
    res = run_bass_kernel_spmd(nc, in_maps, list(range(N_CORES)), trace=trace)
    out = np.concatenate([res.results[c]["Y"] for c in range(N_CORES)], axis=1)
    return out, res


def kernel(**inputs) -> np.ndarray:
    out, _ = _run(inputs, trace=False)
    return out


# revision 28
# speedup vs baseline: 1.1127x; 1.0286x over previous
"""GRU kernel for Trainium2 (8 NeuronCores, data-parallel over batch).

Problem: nn_GRU — X [256, 512, 128] f32, W_z/W_r/W_c [256, 384], b_* [256].
Output: h_history [512, 256, 256] f32.

Sharding: batch 256 -> 8 cores x 32. Each core runs an independent GRU
recurrence over its batch shard; weights replicated; no collectives.

Design (latency-oriented: the 512-step recurrence is serial):
  - bf16 matmul operands, fp32 PSUM accumulation.
  - h_t is carried as the pair (v_t, mu_t) with v = z*c, mu = (z-1)*h_prev,
    h = v - mu. The recurrence matmuls consume v and mu directly (mu through
    negated weight copies), so the h-combine leaves the critical path.
  - r-gate sigmoid is a single fused custom DVE op (deg-7 odd minimax of
    sigma-0.5; r preacts stay within its fit range), followed by one
    scalar_tensor_tensor for rh = (r'+0.5)*h. The Activation engine only
    handles the z-sigmoid and candidate tanh (exact, off/late path).
  - Biases enter PSUM via tiny diag(b) @ ones matmuls; per-step x
    contributions are small per-step matmuls against a pre-transposed,
    pre-bf16 X tile (no separate projection pipeline).
  - Output: h stored [h_low(part), (t, b, hc)]-friendly layout, PE-transposed
    per 2 steps, PSUM->SBUF f32 copy on GPSIMD, single DMA per 2 timesteps.
"""

import os
import sys
from contextlib import ExitStack

sys.path.insert(0, "/opt/trn_rl_repo")

import numpy as np

_NO_OUT = os.environ.get("GRU_NO_OUT", "0") == "1"      # timing exp only
_NO_XSTAGE = os.environ.get("GRU_NO_XSTAGE", "0") == "1"  # timing exp only
_ABL = os.environ.get("GRU_ABL", "")  # comma list: sigz,tanh,r,mu,hcomb

import concourse.bass as bass
import concourse.mybir as mybir
import concourse.tile as tile
from concourse import bacc
from concourse.bass_utils import run_bass_kernel_spmd
from concourse.masks import make_identity

F32 = mybir.dt.float32
BF16 = mybir.dt.bfloat16
AF = mybir.ActivationFunctionType
ALU = mybir.AluOpType

N_CORES = 8
B = 32          # batch per core
S = 512         # sequence length
I = 128         # input features
H = 256         # hidden features
TC = 64         # timesteps per chunk
NCHUNK = S // TC
P = 128

# sigma(x)-0.5 ~= x*(((q3*y + q2)*y + q1)*y + q0), y = x^2 (fit |x|<=5.6)
QS = [0.2402757172521943, -0.014026883800149477, 0.0005286261541401549,
      -7.71991008873346e-06]

_CACHED_NC = None


def _register_sig7():
    """Define + register the fused sigmoid custom DVE op (idempotent)."""
    import concourse.dve_ops as dve_ops
    from concourse.dve_ops import DveOp
    from concourse.dve_spec import (
        C0, C1, C2, C3, Spec, Src0, _has_src1, _spill_c3_to_src1, lower, sq,
    )
    from concourse.dve_uop import DveOpSpec

    for op in dve_ops.OPS:
        if op.name == "ANT_GRU_SIG7":
            return op

    y = sq(Src0)
    body = Src0 * (((C3 * y + C2) * y + C1) * y + C0)

    def ref(in0, in1, s0, s1, imm2):
        yy = in0 * in0
        return (in0 * (((in1 * yy + imm2) * yy + s1) * yy + s0)).astype(
            np.float32
        )

    spec = Spec(body=_spill_c3_to_src1(body), reference=ref)
    uops = lower(spec, ver="v3")
    sha = DveOpSpec(
        name="ANT_GRU_SIG7", opcode=0, uops=uops, rd1_en=_has_src1(spec)
    ).sha("v3")
    op = DveOp("ANT_GRU_SIG7", spec, subdim=False, uops_sha={"v3": sha})
    dve_ops.OPS.append(op)
    dve_ops._SUB_OPCODE_FOR_NAME[op.name] = (
        dve_ops._CUSTOM_DVE_ROW_BASE + len(dve_ops.OPS) - 1
    )
    dve_ops.CUSTOM_DVE_SPECS[op.name] = op.spec
    return op


def _build_nc():
    sig7 = _register_sig7()
    nc = bacc.Bacc(
        "TRN2",
        target_bir_lowering=False,
        debug=False,
        enable_asserts=False,
        num_devices=N_CORES,
    )

    X = nc.dram_tensor("X", [B, S, I], F32, kind="ExternalInput").ap()
    Ws = [
        nc.dram_tensor(n, [H, H + I], F32, kind="ExternalInput").ap()
        for n in ("W_z", "W_r", "W_c")
    ]
    bs = [
        nc.dram_tensor(n, [H], F32, kind="ExternalInput").ap()
        for n in ("b_z", "b_r", "b_c")
    ]
    Y = nc.dram_tensor("Y", [S, B, H], F32, kind="ExternalOutput").ap()

    with tile.TileContext(nc) as tc, ExitStack() as ctx:
        _emit(nc, tc, ctx, sig7, X, Ws, bs, Y)

    nc.compile()
    return nc


def _emit(nc, tc, ctx, sig7, X, Ws, bs, Y):
    const = ctx.enter_context(tc.tile_pool(name="const", bufs=1))
    wtmp_pool = ctx.enter_context(tc.tile_pool(name="wtmp", bufs=2))
    xnpool = ctx.enter_context(tc.tile_pool(name="xn", bufs=2))
    xtpool = ctx.enter_context(tc.tile_pool(name="xt", bufs=2))
    hpool = ctx.enter_context(tc.tile_pool(name="hh", bufs=2))
    rppool = ctx.enter_context(tc.tile_pool(name="rp", bufs=2))
    rhpool = ctx.enter_context(tc.tile_pool(name="rh", bufs=2))
    mupool = ctx.enter_context(tc.tile_pool(name="mu", bufs=2))
    vpool = ctx.enter_context(tc.tile_pool(name="vv", bufs=2))
    zpool = ctx.enter_context(tc.tile_pool(name="zz", bufs=2))
    tpool = ctx.enter_context(tc.tile_pool(name="tt", bufs=2))
    opool = ctx.enter_context(tc.tile_pool(name="ost", bufs=2))
    ppool_t = ctx.enter_context(tc.tile_pool(name="pt", bufs=1, space="PSUM"))
    ppool_r = ctx.enter_context(tc.tile_pool(name="ppr", bufs=2, space="PSUM"))
    ppool_z = ctx.enter_context(tc.tile_pool(name="ppz", bufs=2, space="PSUM"))
    ppool_c = ctx.enter_context(tc.tile_pool(name="ppc", bufs=2, space="PSUM"))

    ident = const.tile([P, P], F32, tag="ident")
    make_identity(nc, ident)
    ident_bf = const.tile([P, P], BF16, tag="identbf")
    nc.scalar.copy(ident_bf, ident)

    # --- weights: lhsT layout [k(part), m] in bf16; negated copies for mu ---
    WT = [[[None] * 3 for _ in range(2)] for _ in range(3)]
    NWT = [[[None] * 2 for _ in range(2)] for _ in range(2)]  # z, r only
    for g in range(3):
        for m in range(2):
            for k in range(3):
                wtmp = wtmp_pool.tile([P, P], F32, tag="wtmp")
                nc.sync.dma_start(
                    wtmp[:], Ws[g][m * P : (m + 1) * P, k * P : (k + 1) * P]
                )
                pt = ppool_t.tile([P, P], F32, tag="pt")
                nc.tensor.transpose(pt, wtmp, ident)
                wl = const.tile([P, P], BF16, tag=f"wl_{g}_{m}_{k}")
                nc.scalar.copy(wl, pt)
                WT[g][m][k] = wl
                if g < 2 and k < 2:
                    nw = const.tile([P, P], BF16, tag=f"nw_{g}_{m}_{k}")
                    nc.vector.tensor_scalar_mul(nw, wl, -1.0)
                    NWT[g][m][k] = nw

    # biases as [128, 2] then diag(b) tiles for the bias matmuls
    diagb = [[None] * 2 for _ in range(3)]
    for g in range(3):
        bt = const.tile([P, 2], F32, tag=f"b_{g}")
        nc.sync.dma_start(bt[:], bs[g].rearrange("(hc p) -> p hc", p=P))
        for m in range(2):
            db = const.tile([P, P], BF16, tag=f"db_{g}_{m}")
            nc.scalar.mul(db, ident, bt[:, m : m + 1])
            diagb[g][m] = db

    ones = const.tile([P, B], BF16, tag="ones")
    nc.vector.memset(ones[:], 1.0)
    q3t = const.tile([P, 1], F32, tag="q3")
    nc.vector.memset(q3t[:], QS[3])
    zero_h = const.tile([P, B, 2], BF16, tag="zh")
    nc.vector.memset(zero_h[:], 0.0)
    zero_v = const.tile([P, B, 2], BF16, tag="zv")
    nc.vector.memset(zero_v[:], 0.0)
    zero_mu = const.tile([P, B, 2], BF16, tag="zmu")
    nc.vector.memset(zero_mu[:], 0.0)

    def emit_x_tile(xt_dst, c, j):
        """Load + transpose X[2j:2j+2, c*TC:(c+1)*TC, :] into xt_dst[:, j]."""
        t0 = c * TC
        xn = xnpool.tile([P, P], F32, tag="xn")
        for boff in range(2):
            nc.sync.dma_start(
                xn[boff * TC : (boff + 1) * TC],
                X[2 * j + boff, t0 : t0 + TC, :],
            )
        pt = ppool_t.tile([P, P], F32, tag="pt")
        nc.tensor.transpose(pt, xn, ident)
        nc.vector.tensor_copy(
            xt_dst[:, j].rearrange("p b t -> p (b t)"), pt
        )

    # chunk 0's x tiles up front; xt layout [p, j, boff, t]
    xt_cur = xtpool.tile([P, 16, 2, TC], BF16, tag="xt")
    for j in range(16):
        emit_x_tile(xt_cur, 0, j)

    h_prev = zero_h[:]
    v_prev = zero_v[:]
    mu_prev = zero_mu[:]

    for c in range(NCHUNK):
        t0 = c * TC
        xt_next = None
        if c + 1 < NCHUNK:
            xt_next = xtpool.tile([P, 16, 2, TC], BF16, tag="xt")
        h_hist = hpool.tile([P, TC, B, 2], BF16, tag="hh")
        # per-chunk rings: fresh slice per step -> no per-step WAR sems
        rp_ring = rppool.tile([P, TC, 2, B], F32, tag="rp")
        rh_ring = rhpool.tile([P, TC, B, 2], BF16, tag="rh")
        mu_ring = mupool.tile([P, TC, B, 2], BF16, tag="mu")
        v_ring = vpool.tile([P, TC, B, 2], BF16, tag="vv")
        z_ring = zpool.tile([P, TC, B, 2], BF16, tag="zz")
        T_ring = tpool.tile([P, TC, B, 2], BF16, tag="tt")
        ost_ring = opool.tile([P, TC // 2, P], F32, tag="ost")

        def emit_prep(xt_tile, s, pg3, mu_rhs):
            """Dependency-free PSUM prep (bias + x + mu) for a step; emitted
            a step early so only v-matmuls trail the critical v arrival."""
            x_rhs = xt_tile[:, :, :, s]
            pr_n, pz_n, pc_n = pg3
            for g, pg in ((1, pr_n), (0, pz_n)):
                for m in range(2):
                    nc.tensor.matmul(pg[:, m], lhsT=diagb[g][m], rhs=ones[:],
                                     start=True, stop=False)
                    nc.tensor.matmul(pg[:, m], lhsT=WT[g][m][2], rhs=x_rhs,
                                     start=False, stop=False)
                    for k in range(2):
                        nc.tensor.matmul(pg[:, m], lhsT=NWT[g][m][k],
                                         rhs=mu_rhs[:, :, k],
                                         start=False, stop=(k == 1),
                                         skip_group_check=True)
            for m in range(2):
                nc.tensor.matmul(pc_n[:, m], lhsT=diagb[2][m], rhs=ones[:],
                                 start=True, stop=False)
                nc.tensor.matmul(pc_n[:, m], lhsT=WT[2][m][2], rhs=x_rhs,
                                 start=False, stop=True,
                                 skip_group_check=True)

        def alloc_psums():
            pr_t = ppool_r.tile([P, 2, B], F32, tag="pr")
            pz_t = ppool_z.tile([P, 2, B], F32, tag="pz")
            pc_t = ppool_c.tile([P, 2, B], F32, tag="pc")
            return (pr_t, pz_t, pc_t)

        if c == 0:
            cur_psums = alloc_psums()
            emit_prep(xt_cur, 0, cur_psums, zero_mu[:])
            _emit.cur_psums = cur_psums
        cur_psums = _emit.cur_psums

        for s in range(TC):
            pr, pz, pc = cur_psums
            # r-gate: only v-matmuls trail the critical arrival
            for m in range(2):
                for k in range(2):
                    nc.tensor.matmul(pr[:, m], lhsT=WT[1][m][k],
                                     rhs=v_prev[:, :, k],
                                     start=False, stop=(k == 1),
                                     skip_group_check=True)
            # r' on DVE (custom fused sigmoid-0.5)
            rp = rp_ring[:, s]
            nc.vector._custom_dve(sig7, out=rp, in0=pr[:], in1=q3t[:],
                                  s0=QS[0], s1=QS[1], imm2=QS[2])
            # z-gate v-matmuls
            for m in range(2):
                for k in range(2):
                    nc.tensor.matmul(pz[:, m], lhsT=WT[0][m][k],
                                     rhs=v_prev[:, :, k],
                                     start=False, stop=(k == 1),
                                     skip_group_check=True)
            # z on Act
            if "sigz" in _ABL:
                z_s = zero_h[:]
            else:
                z_s = z_ring[:, s]
                nc.scalar.activation(z_s, pz.rearrange("p m b -> p b m"),
                                     AF.Sigmoid)
            # rh = (r' + 0.5) * h_prev
            if "r" in _ABL.split(","):
                rh = h_prev
            else:
                rh = rh_ring[:, s]
                nc.vector.scalar_tensor_tensor(
                    rh, rp.rearrange("p m b -> p b m"), 0.5, h_prev,
                    ALU.add, ALU.mult,
                )
            # candidate rh-matmuls
            for m in range(2):
                for k in range(2):
                    nc.tensor.matmul(pc[:, m], lhsT=WT[2][m][k],
                                     rhs=rh[:, :, k],
                                     start=False, stop=(k == 1),
                                     skip_group_check=True)
            # mu_s = (z - 1) * h_prev
            if "mu" in _ABL:
                mu_s = zero_mu[:]
            else:
                mu_s = mu_ring[:, s]
                nc.vector.scalar_tensor_tensor(
                    mu_s, z_s, 1.0, h_prev, ALU.subtract, ALU.mult,
                )
            # prep next step's psums (executes during the tanh window)
            nxt = None
            if s + 1 < TC:
                nxt = alloc_psums()
                emit_prep(xt_cur, s + 1, nxt, mu_s)
            elif xt_next is not None:
                nxt = alloc_psums()
                emit_prep(xt_next, 0, nxt, mu_s)
            _emit.cur_psums = nxt
            cur_psums = nxt
            # tanh on Act
            if "tanh" in _ABL:
                T_s = z_s
            else:
                T_s = T_ring[:, s]
                nc.scalar.activation(T_s, pc.rearrange("p m b -> p b m"),
                                     AF.Tanh)
            # v_s = z * T ; h_s = v - mu
            v_s = v_ring[:, s]
            nc.vector.tensor_mul(v_s, z_s, T_s)
            nc.vector.tensor_sub(h_hist[:, s], v_s, mu_s)

            h_prev = h_hist[:, s]
            v_prev = v_s
            mu_prev = mu_s

            # output transpose + DMA every 2 steps
            if s % 2 == 1 and not _NO_OUT:
                ptb = ppool_t.tile([P, P], BF16, tag="ptb")
                nc.tensor.transpose(
                    ptb,
                    h_hist[:, s - 1 : s + 1].rearrange(
                        "p t b hc -> p (t b hc)"
                    ),
                    ident_bf,
                )
                ost = ost_ring[:, s // 2]
                nc.scalar.copy(ost, ptb)
                nc.sync.dma_start(
                    Y[t0 + s - 1 : t0 + s + 1, :, :].rearrange(
                        "t b (hc hl) -> (t b hc) hl", hc=2
                    ),
                    ost,
                )

            # stage next chunk's x tiles (1 per 4 steps)
            if xt_next is not None and s % 4 == 0 and not _NO_XSTAGE:
                emit_x_tile(xt_next, c + 1, s // 4)

        if xt_next is not None and _NO_XSTAGE:
            for j in range(16):
                emit_x_tile(xt_next, c + 1, j)
        xt_cur = xt_next


def _get_nc():
    global _CACHED_NC
    if _CACHED_NC is None:
        _CACHED_NC = _build_nc()
    return _CACHED_NC


def _run(inputs, trace=False):
    nc = _get_nc()
    X = np.ascontiguousarray(np.asarray(inputs["X"], dtype=np.float32))
    names = ("W_z", "b_z", "W_r", "b_r", "W_c", "b_c")
    shared = {
        n: np.ascontiguousarray(np.asarray(inputs[n], dtype=np.float32))
        for n in names
    }
    in_maps = []
    for core in range(N_CORES):
        m = {"X": np.ascontiguousarray(X[core * B : (core + 1) * B])}
        m.update(shared)
        in_maps.append(m)
    res = run_bass_kernel_spmd(nc, in_maps, list(range(N_CORES)), trace=trace)
    out = np.concatenate([res.results[c]["Y"] for c in range(N_CORES)], axis=1)
    return out, res


def kernel(**inputs) -> np.ndarray:
    out, _ = _run(inputs, trace=False)
    return out
